# revision 2
# baseline (speedup 1.0000x reference)
"""MoE-LoRA Linear kernel for 8 Trainium2 NeuronCores — bf16 single-pass.

Sharding: core c -> (batch b = c//2, out-feature half = c%2).
Each core computes out[b, :, half] = x[b] @ W_half.T + b_half
                                   + SCALING * router-weighted LoRA.

v2 vs v1: all matmul operands bf16 (same PE rate as f32r, half the DMA),
x kept fully SBUF-resident (no panel reload), W streamed exactly once,
output written exactly once (no DMA-accumulate phase), and the router
logits ride for free in the LoRA down-projection matmul: the stationary
operand is [router_W; lora_A] (72 rows), so psum rows 0-7 are per-token
router logits and a cheap [8,512] vector reduce replaces the whole
mean-pool + router-matmul chain. Row order keeps every compute op
partition-aligned (engines cannot shift partitions on HW); the lora
matmul contracts over 73 rows with zeros in Bta rows 0-7.

Device layout (per core):
  x4   [4, 128, 32, 512] bf16  x[b].T as chunk-major [tch][p][dt][t]
  W8   [8, 128, 32, 256] bf16  W_half.T as chunk-major [och][p][dt][o]
  Ar   [128, 32, 72]     bf16  [router_W (8 rows); lora_A (64 rows)] as [p][dt][r]
  Bta  [73, 2048]        bf16  rows 0-7: zero; 8-71: lora_B[half] as [er, o]; row 72: b_base
  out  [2048, 2048]      f32   result transposed: [o, t]

Main matmul: psum[o128, t512] += W_tile[d128, o128].T @ x[d128, t512]
over 32 d-tiles, then one K=73 matmul adds router-weighted LoRA + bias
(row 72 of Bta is the bias, matched by a ones-row in the augmented h).
"""
import sys

sys.path.insert(0, "/opt/trn_rl_repo")

import numpy as np
import ml_dtypes

import concourse.bass as bass
import concourse.mybir as mybir
import concourse.tile as tile
from concourse import bacc, bass_isa
from concourse.bass_utils import run_bass_kernel_spmd

F32 = mybir.dt.float32
BF16 = mybir.dt.bfloat16

D, T, O_SH, E, R = 4096, 2048, 2048, 8, 8
ER = E * R  # 64
AR = ER + E  # 72: lora_A rows + router_W rows
DT = D // 128  # 32 d-tiles
TCH = 512  # t-chunk
NT = T // TCH  # 4 t-chunks
OCH = 256  # o-chunk (per W DMA; 2 o-tiles)
NO = O_SH // OCH  # 8 W chunks
OT = O_SH // 128  # 16 o-tiles
ROUTER_TEMP = 1.0
SCALING = 16.0 / 8.0

_nc_cache = []


def build():
    nc = bacc.Bacc(None, target_bir_lowering=False)
    x4 = nc.dram_tensor("x4", [NT, 128, DT, TCH], BF16, kind="ExternalInput")
    W8 = nc.dram_tensor("W8", [NO, 128, DT, OCH], BF16, kind="ExternalInput")
    Ar = nc.dram_tensor("Ar", [128, DT, AR], BF16, kind="ExternalInput")
    Bta = nc.dram_tensor("Bta", [AR + 1, O_SH], BF16, kind="ExternalInput")
    rb = nc.dram_tensor("rb", [E], F32, kind="ExternalInput")
    ones_d = nc.dram_tensor("ones_d", [T], BF16, kind="ExternalInput")
    wconst = nc.dram_tensor("wconst", [AR + 1], BF16, kind="ExternalInput")
    out = nc.dram_tensor("out", [O_SH, T], F32, kind="ExternalOutput")
    wscratch = nc.dram_tensor("wscratch", [E], BF16)

    with tile.TileContext(nc) as tc:
        with (
            tc.tile_pool(name="single", bufs=1) as single,
            tc.tile_pool(name="wpool", bufs=2) as wpool,
            tc.tile_pool(name="ev", bufs=4) as evpool,
            tc.tile_pool(name="psp", bufs=2, space="PSUM") as psp,
            tc.tile_pool(name="psm", bufs=6, space="PSUM") as psm,
        ):
            # ---- persistent tiles ----
            atp = single.tile([128, DT, AR], BF16)
            xfull = single.tile([128, NT, DT, TCH], BF16)
            haug = single.tile([AR + 1, T], BF16)
            bta = single.tile([AR + 1, O_SH], BF16)
            rb8 = single.tile([E, 1], F32)
            ylg = single.tile([E, NT], F32)

            # ---- DMA queues ----
            # One queue (sync/SP) carries every startup-critical load in
            # exact priority order — concurrent queues starve each other.
            # x0 goes in contiguous dt-quarters (each h matmul gates only
            # on the quarter holding its d-tile) with W0 slotted before
            # the last quarter so m(o0,*) can start right after h(t0).
            # gpsimd (Pool): small router tensors (bta last: needed
            # latest), later half the out DMAs.
            nc.gpsimd.dma_start(rb8[:], rb[:, None])
            nc.gpsimd.dma_start(haug[AR : AR + 1, :], ones_d[None, :])
            nc.gpsimd.dma_start(bta[:], Bta[:])

            wtiles = []
            nc.sync.dma_start(atp[:], Ar[:])
            for q in range(3):
                nc.sync.dma_start(
                    xfull[:, 0, q * 8 : (q + 1) * 8, :], x4[0][:, q * 8 : (q + 1) * 8]
                )
            wt = wpool.tile([128, DT, OCH], BF16, tag="wt")
            nc.sync.dma_start(wt[:, 0:16], W8[0][:, 0:16])
            nc.sync.dma_start(wt[:, 16:32], W8[0][:, 16:32])
            wtiles.append(wt)
            nc.sync.dma_start(xfull[:, 0, 24:32, :], x4[0][:, 24:32])
            nc.sync.dma_start(xfull[:, 1], x4[1])
            wt = wpool.tile([128, DT, OCH], BF16, tag="wt")
            nc.sync.dma_start(wt[:], W8[1])
            wtiles.append(wt)
            nc.sync.dma_start(xfull[:, 2], x4[2])
            nc.sync.dma_start(xfull[:, 3], x4[3])
            # W2..W7 go on the scalar queue; each is pool-gated on the
            # buffer it rotates into, so none of them compete with the
            # startup stream above.

            # ---- HAM warm-up: the PE idles ~14us waiting for x0; run
            # dummy matmuls on a zeroed tile so the clock gate is already
            # at 8/8 when the first real matmul issues.
            warm = single.tile([128, 256], BF16)
            nc.vector.memset(warm[:], 0.0)
            wps = psp.tile([128, 256], F32, tag="hps", name="warmps")
            for i in range(32):
                nc.tensor.matmul(
                    wps[:], warm[:, 0:128], warm[:], start=True, stop=True
                )

            # ---- emission helpers ----
            def h_chunk(c, pieces=1):
                hps = psp.tile([AR, TCH], F32, tag="hps")
                w = TCH // pieces
                for p in range(pieces):
                    csl = slice(p * w, (p + 1) * w)
                    for d in range(DT):
                        nc.tensor.matmul(
                            hps[:, csl],
                            atp[:, d, :],
                            xfull[:, c, d, csl],
                            start=(d == 0),
                            stop=(d == DT - 1),
                        )
                nc.vector.tensor_copy(haug[0:AR, c * TCH : (c + 1) * TCH], hps[:])
                nc.vector.reduce_sum(
                    ylg[:, c : c + 1], hps[0:E, :], axis=mybir.AxisListType.X
                )

            def main_mm(o, c, ps):
                wt = wtiles[o // 2]
                osl = slice((o % 2) * 128, (o % 2) * 128 + 128)
                for d in range(DT):
                    nc.tensor.matmul(
                        ps[:],
                        wt[:, d, osl],
                        xfull[:, c, d, :],
                        start=(d == 0),
                        stop=False,
                    )

            def lora_mm(o, c, ps):
                nc.tensor.matmul(
                    ps[:],
                    bta[:, o * 128 : (o + 1) * 128],
                    haug[:, c * TCH : (c + 1) * TCH],
                    start=False,
                    stop=True,
                )
                ev = evpool.tile([128, TCH], F32, tag="ev")
                nc.vector.tensor_copy(ev[:], ps[:])
                eng = nc.gpsimd if (o * NT + c) % 2 == 0 else nc.sync
                eng.dma_start(
                    out[o * 128 : (o + 1) * 128, c * TCH : (c + 1) * TCH], ev[:]
                )

            def router():
                # logits = sum_t(rW^T x)[e] / T + rb;  weights = softmax * SCALING
                lg8 = single.tile([E, 1], F32)
                nc.vector.reduce_sum(lg8[:], ylg[:], axis=mybir.AxisListType.X)
                nc.scalar.activation(
                    lg8[:], lg8[:], mybir.ActivationFunctionType.Copy,
                    scale=1.0 / (T * ROUTER_TEMP),
                )
                nc.vector.tensor_tensor(lg8[:], lg8[:], rb8[:], mybir.AluOpType.add)
                m8 = single.tile([E, 1], F32)
                nc.gpsimd.partition_all_reduce(
                    m8[:], lg8[:], channels=E, reduce_op=bass_isa.ReduceOp.max
                )
                e8 = single.tile([E, 1], F32)
                nc.vector.tensor_tensor(e8[:], lg8[:], m8[:], mybir.AluOpType.subtract)
                nc.scalar.activation(e8[:], e8[:], mybir.ActivationFunctionType.Exp)
                s8 = single.tile([E, 1], F32)
                nc.gpsimd.partition_all_reduce(
                    s8[:], e8[:], channels=E, reduce_op=bass_isa.ReduceOp.add
                )
                r8 = single.tile([E, 1], F32)
                nc.vector.reciprocal(r8[:], s8[:])
                w8 = single.tile([E, 1], F32)
                nc.vector.tensor_tensor(w8[:], e8[:], r8[:], mybir.AluOpType.mult)
                nc.vector.tensor_scalar_mul(w8[:], w8[:], SCALING)
                w8b = single.tile([E, 1], BF16)
                nc.vector.tensor_copy(w8b[:], w8[:])
                wexp = single.tile([AR + 1, 1], BF16)
                nc.gpsimd.dma_start(wexp[:], wconst[:, None])
                wsrc = bass.AP(tensor=w8b[:].tensor, offset=0, ap=[[1, E], [0, R]])
                nc.gpsimd.dma_start(wexp[E:AR, :], wsrc)
                # scale Bta rows by router weight (rows 0-7 stay 0, row
                # 72 *= 1.0); per-column-chunk so lora(0,0) gates only on
                # the first slice.
                for cs in range(4):
                    csl = slice(cs * 512, (cs + 1) * 512)
                    nc.vector.tensor_tensor(
                        bta[:, csl], bta[:, csl],
                        wexp[:].to_broadcast([AR + 1, 512]),
                        mybir.AluOpType.mult,
                    )

            # ---- startup group: o-tiles 0,1 chunk-major. All PE work up
            # to h1 needs only x0+W0, which bridges the bandwidth-bound
            # x1..x3 stream; h3 runs right after h2 so the router pipeline
            # (y-reduce, softmax, Bta scale) completes before lora(0,0).
            ps_g = {}

            def open_main(o, c):
                ps_g[(o, c)] = psm.tile([128, TCH], F32, tag="main", name=f"m{o}_{c}")
                main_mm(o, c, ps_g[(o, c)])

            h_chunk(0)
            open_main(0, 0)
            open_main(1, 0)
            h_chunk(1)
            open_main(0, 1)
            open_main(1, 1)
            h_chunk(2)
            h_chunk(3)
            router()
            open_main(0, 2)
            open_main(1, 2)
            lora_mm(0, 0, ps_g[(0, 0)])
            lora_mm(1, 0, ps_g[(1, 0)])
            open_main(0, 3)
            open_main(1, 3)
            lora_mm(0, 1, ps_g[(0, 1)])
            lora_mm(1, 1, ps_g[(1, 1)])
            lora_mm(0, 2, ps_g[(0, 2)])
            lora_mm(1, 2, ps_g[(1, 2)])
            lora_mm(0, 3, ps_g[(0, 3)])
            lora_mm(1, 3, ps_g[(1, 3)])

            # ---- steady state ----
            for o in range(2, OT):
                if o % 2 == 0 and o // 2 + 1 < NO:
                    wt = wpool.tile([128, DT, OCH], BF16, tag="wt")
                    nc.scalar.dma_start(wt[:], W8[o // 2 + 1])
                    wtiles.append(wt)
                for c in range(NT):
                    ps = psm.tile([128, TCH], F32, tag="main", name=f"m{o}_{c}")
                    main_mm(o, c, ps)
                    lora_mm(o, c, ps)
    nc.compile()
    return nc


def _get_nc():
    if not _nc_cache:
        _nc_cache.append(build())
    return _nc_cache[0]


def _bf16(a):
    return np.ascontiguousarray(a.astype(ml_dtypes.bfloat16))


def _prep_in_maps(x, W_base, b_base, lora_A, lora_B, router_W, router_b):
    Arows = np.concatenate([router_W, lora_A.reshape(ER, D)], axis=0)  # [72, D]
    Ar_h = _bf16(Arows.T.reshape(DT, 128, AR).transpose(1, 0, 2))  # [128, 32, 72]
    in_maps = []
    for c in range(8):
        b, half = c // 2, c % 2
        osl = slice(half * O_SH, (half + 1) * O_SH)
        xT = x[b].T  # [D, T]
        x4_h = _bf16(xT.reshape(DT, 128, NT, TCH).transpose(2, 1, 0, 3))
        Wt = W_base[osl].T  # [D, O_SH]
        W8_h = _bf16(Wt.reshape(DT, 128, NO, OCH).transpose(2, 1, 0, 3))
        Bt = lora_B[:, osl, :].transpose(0, 2, 1).reshape(ER, O_SH)
        Bta_h = _bf16(
            np.concatenate(
                [np.zeros((E, O_SH), np.float32), Bt, b_base[osl][None, :]], axis=0
            )
        )
        in_maps.append(
            {
                "x4": x4_h,
                "W8": W8_h,
                "Ar": Ar_h,
                "Bta": Bta_h,
                "rb": router_b.astype(np.float32),
                "ones_d": _bf16(np.ones(T, np.float32)),
                "wconst": _bf16(
                    np.concatenate([np.zeros(AR, np.float32), np.ones(1, np.float32)])
                ),
            }
        )
    return in_maps


def kernel(x, W_base, b_base, lora_A, lora_B, router_W, router_b):
    x = np.asarray(x, dtype=np.float32)
    W_base = np.asarray(W_base, dtype=np.float32)
    b_base = np.asarray(b_base, dtype=np.float32)
    lora_A = np.asarray(lora_A, dtype=np.float32)
    lora_B = np.asarray(lora_B, dtype=np.float32)
    router_W = np.asarray(router_W, dtype=np.float32)
    router_b = np.asarray(router_b, dtype=np.float32)

    B, S, D_ = x.shape
    O = W_base.shape[0]
    in_maps = _prep_in_maps(x, W_base, b_base, lora_A, lora_B, router_W, router_b)

    global _last_in_maps
    _last_in_maps = in_maps
    nc = _get_nc()
    res = run_bass_kernel_spmd(nc, in_maps, core_ids=list(range(8)))
    out = np.empty((B, S, O), dtype=np.float32)
    for c in range(8):
        b, half = c // 2, c % 2
        out[b, :, half * O_SH : (half + 1) * O_SH] = res.results[c]["out"].T
    return out


# revision 3
# speedup vs baseline: 1.0150x; 1.0150x over previous
"""MoE-LoRA Linear kernel for 8 Trainium2 NeuronCores — bf16 single-pass.

Sharding: core c -> (batch b = c//2, out-feature half = c%2).
Each core computes out[b, :, half] = x[b] @ W_half.T + b_half
                                   + SCALING * router-weighted LoRA.

v2 vs v1: all matmul operands bf16 (same PE rate as f32r, half the DMA),
x kept fully SBUF-resident (no panel reload), W streamed exactly once,
output written exactly once (no DMA-accumulate phase), and the router
logits ride for free in the LoRA down-projection matmul: the stationary
operand is [router_W; lora_A] (72 rows), so psum rows 0-7 are per-token
router logits and a cheap [8,512] vector reduce replaces the whole
mean-pool + router-matmul chain. Row order keeps every compute op
partition-aligned (engines cannot shift partitions on HW); the lora
matmul contracts over 73 rows with zeros in Bta rows 0-7.

Device layout (per core):
  x4   [4, 128, 32, 512] bf16  x[b].T as chunk-major [tch][p][dt][t]
  W8   [8, 128, 32, 256] bf16  W_half.T as chunk-major [och][p][dt][o]
  Ar   [128, 32, 72]     bf16  [router_W (8 rows); lora_A (64 rows)] as [p][dt][r]
  Bta  [73, 2048]        bf16  rows 0-7: zero; 8-71: lora_B[half] as [er, o]; row 72: b_base
  out  [2048, 2048]      f32   result transposed: [o, t]

Main matmul: psum[o128, t512] += W_tile[d128, o128].T @ x[d128, t512]
over 32 d-tiles, then one K=73 matmul adds router-weighted LoRA + bias
(row 72 of Bta is the bias, matched by a ones-row in the augmented h).
"""
import sys

sys.path.insert(0, "/opt/trn_rl_repo")

import numpy as np
import ml_dtypes

import concourse.bass as bass
import concourse.mybir as mybir
import concourse.tile as tile
from concourse import bacc, bass_isa
from concourse.bass_utils import run_bass_kernel_spmd

F32 = mybir.dt.float32
BF16 = mybir.dt.bfloat16

D, T, O_SH, E, R = 4096, 2048, 2048, 8, 8
ER = E * R  # 64
AR = ER + E  # 72: lora_A rows + router_W rows
DT = D // 128  # 32 d-tiles
TCH = 512  # t-chunk
NT = T // TCH  # 4 t-chunks
OCH = 256  # o-chunk (per W DMA; 2 o-tiles)
NO = O_SH // OCH  # 8 W chunks
OT = O_SH // 128  # 16 o-tiles
ROUTER_TEMP = 1.0
SCALING = 16.0 / 8.0

_nc_cache = []


def build():
    nc = bacc.Bacc(None, target_bir_lowering=False)
    x4 = nc.dram_tensor("x4", [NT, 128, DT, TCH], BF16, kind="ExternalInput")
    W8 = nc.dram_tensor("W8", [NO, 128, DT, OCH], BF16, kind="ExternalInput")
    Ar = nc.dram_tensor("Ar", [128, DT, AR], BF16, kind="ExternalInput")
    Bta = nc.dram_tensor("Bta", [AR + 1, O_SH], BF16, kind="ExternalInput")
    rb = nc.dram_tensor("rb", [E], F32, kind="ExternalInput")
    ones_d = nc.dram_tensor("ones_d", [T], BF16, kind="ExternalInput")
    wconst = nc.dram_tensor("wconst", [AR + 1], BF16, kind="ExternalInput")
    out = nc.dram_tensor("out", [O_SH, T], F32, kind="ExternalOutput")
    wscratch = nc.dram_tensor("wscratch", [E], BF16)

    with tile.TileContext(nc) as tc:
        with (
            tc.tile_pool(name="single", bufs=1) as single,
            tc.tile_pool(name="wpool", bufs=2) as wpool,
            tc.tile_pool(name="ev", bufs=4) as evpool,
            tc.tile_pool(name="psp", bufs=2, space="PSUM") as psp,
            tc.tile_pool(name="psm", bufs=6, space="PSUM") as psm,
        ):
            # ---- persistent tiles ----
            atp = single.tile([128, DT, AR], BF16)
            xfull = single.tile([128, NT, DT, TCH], BF16)
            haug = single.tile([AR + 1, T], BF16)
            bta = single.tile([AR + 1, O_SH], BF16)
            rb8 = single.tile([E, 1], F32)
            ylg = single.tile([E, NT], F32)

            # ---- DMA queues ----
            # One queue (sync/SP) carries every startup-critical load in
            # exact priority order — concurrent queues starve each other.
            # x0 goes in contiguous dt-quarters (each h matmul gates only
            # on the quarter holding its d-tile) with W0 slotted before
            # the last quarter so m(o0,*) can start right after h(t0).
            # gpsimd (Pool): small router tensors (bta last: needed
            # latest), later half the out DMAs.
            nc.gpsimd.dma_start(rb8[:], rb[:, None])
            nc.gpsimd.dma_start(haug[AR : AR + 1, :], ones_d[None, :])
            nc.gpsimd.dma_start(bta[:], Bta[:])

            wtiles = []
            nc.sync.dma_start(atp[:], Ar[:])
            for q in range(3):
                nc.sync.dma_start(
                    xfull[:, 0, q * 8 : (q + 1) * 8, :], x4[0][:, q * 8 : (q + 1) * 8]
                )
            wt = wpool.tile([128, DT, OCH], BF16, tag="wt")
            nc.sync.dma_start(wt[:, 0:16], W8[0][:, 0:16])
            nc.sync.dma_start(wt[:, 16:32], W8[0][:, 16:32])
            wtiles.append(wt)
            nc.sync.dma_start(xfull[:, 0, 24:32, :], x4[0][:, 24:32])
            nc.sync.dma_start(xfull[:, 1, 0:16, :], x4[1][:, 0:16])
            nc.sync.dma_start(xfull[:, 1, 16:32, :], x4[1][:, 16:32])
            wt = wpool.tile([128, DT, OCH], BF16, tag="wt")
            nc.sync.dma_start(wt[:], W8[1])
            wtiles.append(wt)
            nc.sync.dma_start(xfull[:, 2, 0:16, :], x4[2][:, 0:16])
            nc.sync.dma_start(xfull[:, 2, 16:32, :], x4[2][:, 16:32])
            nc.sync.dma_start(xfull[:, 3, 0:16, :], x4[3][:, 0:16])
            nc.sync.dma_start(xfull[:, 3, 16:32, :], x4[3][:, 16:32])
            # W2..W7 go on the scalar queue; each is pool-gated on the
            # buffer it rotates into, so none of them compete with the
            # startup stream above.

            # ---- HAM warm-up: the PE idles ~14us waiting for x0; run
            # dummy matmuls on a zeroed tile so the clock gate is already
            # at 8/8 when the first real matmul issues.
            warm = single.tile([128, 256], BF16)
            nc.vector.memset(warm[:], 0.0)
            wps = psp.tile([128, 256], F32, tag="hps", name="warmps")
            for i in range(40):
                nc.tensor.matmul(
                    wps[:], warm[:, 0:128], warm[:], start=True, stop=True
                )

            # ---- emission helpers ----
            def h_chunk(c, pieces=1):
                hps = psp.tile([AR, TCH], F32, tag="hps")
                w = TCH // pieces
                for p in range(pieces):
                    csl = slice(p * w, (p + 1) * w)
                    for d in range(DT):
                        nc.tensor.matmul(
                            hps[:, csl],
                            atp[:, d, :],
                            xfull[:, c, d, csl],
                            start=(d == 0),
                            stop=(d == DT - 1),
                        )
                nc.vector.tensor_copy(haug[0:AR, c * TCH : (c + 1) * TCH], hps[:])
                nc.vector.reduce_sum(
                    ylg[:, c : c + 1], hps[0:E, :], axis=mybir.AxisListType.X
                )

            def main_mm(o, c, ps):
                wt = wtiles[o // 2]
                osl = slice((o % 2) * 128, (o % 2) * 128 + 128)
                for d in range(DT):
                    nc.tensor.matmul(
                        ps[:],
                        wt[:, d, osl],
                        xfull[:, c, d, :],
                        start=(d == 0),
                        stop=False,
                    )

            def lora_mm(o, c, ps):
                nc.tensor.matmul(
                    ps[:],
                    bta[:, o * 128 : (o + 1) * 128],
                    haug[:, c * TCH : (c + 1) * TCH],
                    start=False,
                    stop=True,
                )
                ev = evpool.tile([128, TCH], F32, tag="ev")
                nc.vector.tensor_copy(ev[:], ps[:])
                eng = nc.gpsimd if (o * NT + c) % 2 == 0 else nc.sync
                eng.dma_start(
                    out[o * 128 : (o + 1) * 128, c * TCH : (c + 1) * TCH], ev[:]
                )

            def router():
                # logits = sum_t(rW^T x)[e] / T + rb;  weights = softmax * SCALING
                lg8 = single.tile([E, 1], F32)
                nc.vector.reduce_sum(lg8[:], ylg[:], axis=mybir.AxisListType.X)
                nc.scalar.activation(
                    lg8[:], lg8[:], mybir.ActivationFunctionType.Copy,
                    scale=1.0 / (T * ROUTER_TEMP),
                )
                nc.vector.tensor_tensor(lg8[:], lg8[:], rb8[:], mybir.AluOpType.add)
                m8 = single.tile([E, 1], F32)
                nc.gpsimd.partition_all_reduce(
                    m8[:], lg8[:], channels=E, reduce_op=bass_isa.ReduceOp.max
                )
                e8 = single.tile([E, 1], F32)
                nc.vector.tensor_tensor(e8[:], lg8[:], m8[:], mybir.AluOpType.subtract)
                nc.scalar.activation(e8[:], e8[:], mybir.ActivationFunctionType.Exp)
                s8 = single.tile([E, 1], F32)
                nc.gpsimd.partition_all_reduce(
                    s8[:], e8[:], channels=E, reduce_op=bass_isa.ReduceOp.add
                )
                r8 = single.tile([E, 1], F32)
                nc.vector.reciprocal(r8[:], s8[:])
                w8 = single.tile([E, 1], F32)
                nc.vector.tensor_tensor(w8[:], e8[:], r8[:], mybir.AluOpType.mult)
                nc.vector.tensor_scalar_mul(w8[:], w8[:], SCALING)
                w8b = single.tile([E, 1], BF16)
                nc.vector.tensor_copy(w8b[:], w8[:])
                wexp = single.tile([AR + 1, 1], BF16)
                nc.gpsimd.dma_start(wexp[:], wconst[:, None])
                wsrc = bass.AP(tensor=w8b[:].tensor, offset=0, ap=[[1, E], [0, R]])
                nc.gpsimd.dma_start(wexp[E:AR, :], wsrc)
                # scale Bta rows by router weight (rows 0-7 stay 0, row
                # 72 *= 1.0); per-column-chunk so lora(0,0) gates only on
                # the first slice.
                for cs in range(4):
                    csl = slice(cs * 512, (cs + 1) * 512)
                    nc.vector.tensor_tensor(
                        bta[:, csl], bta[:, csl],
                        wexp[:].to_broadcast([AR + 1, 512]),
                        mybir.AluOpType.mult,
                    )

            # ---- startup group: o-tiles 0,1 chunk-major. All PE work up
            # to h1 needs only x0+W0, which bridges the bandwidth-bound
            # x1..x3 stream; h3 runs right after h2 so the router pipeline
            # (y-reduce, softmax, Bta scale) completes before lora(0,0).
            ps_g = {}

            def open_main(o, c):
                ps_g[(o, c)] = psm.tile([128, TCH], F32, tag="main", name=f"m{o}_{c}")
                main_mm(o, c, ps_g[(o, c)])

            h_chunk(0)
            open_main(0, 0)
            open_main(1, 0)
            h_chunk(1)
            open_main(0, 1)
            open_main(1, 1)
            h_chunk(2)
            h_chunk(3)
            router()
            open_main(0, 2)
            open_main(1, 2)
            lora_mm(0, 0, ps_g[(0, 0)])
            lora_mm(1, 0, ps_g[(1, 0)])
            open_main(0, 3)
            open_main(1, 3)
            lora_mm(0, 1, ps_g[(0, 1)])
            lora_mm(1, 1, ps_g[(1, 1)])
            lora_mm(0, 2, ps_g[(0, 2)])
            lora_mm(1, 2, ps_g[(1, 2)])
            lora_mm(0, 3, ps_g[(0, 3)])
            lora_mm(1, 3, ps_g[(1, 3)])

            # ---- steady state ----
            for o in range(2, OT):
                if o % 2 == 0 and o // 2 + 1 < NO:
                    wt = wpool.tile([128, DT, OCH], BF16, tag="wt")
                    nc.scalar.dma_start(wt[:], W8[o // 2 + 1])
                    wtiles.append(wt)
                for c in range(NT):
                    ps = psm.tile([128, TCH], F32, tag="main", name=f"m{o}_{c}")
                    main_mm(o, c, ps)
                    lora_mm(o, c, ps)
    nc.compile()
    return nc


def _get_nc():
    if not _nc_cache:
        _nc_cache.append(build())
    return _nc_cache[0]


def _bf16(a):
    return np.ascontiguousarray(a.astype(ml_dtypes.bfloat16))


def _prep_in_maps(x, W_base, b_base, lora_A, lora_B, router_W, router_b):
    Arows = np.concatenate([router_W, lora_A.reshape(ER, D)], axis=0)  # [72, D]
    Ar_h = _bf16(Arows.T.reshape(DT, 128, AR).transpose(1, 0, 2))  # [128, 32, 72]
    in_maps = []
    for c in range(8):
        b, half = c // 2, c % 2
        osl = slice(half * O_SH, (half + 1) * O_SH)
        xT = x[b].T  # [D, T]
        x4_h = _bf16(xT.reshape(DT, 128, NT, TCH).transpose(2, 1, 0, 3))
        Wt = W_base[osl].T  # [D, O_SH]
        W8_h = _bf16(Wt.reshape(DT, 128, NO, OCH).transpose(2, 1, 0, 3))
        Bt = lora_B[:, osl, :].transpose(0, 2, 1).reshape(ER, O_SH)
        Bta_h = _bf16(
            np.concatenate(
                [np.zeros((E, O_SH), np.float32), Bt, b_base[osl][None, :]], axis=0
            )
        )
        in_maps.append(
            {
                "x4": x4_h,
                "W8": W8_h,
                "Ar": Ar_h,
                "Bta": Bta_h,
                "rb": router_b.astype(np.float32),
                "ones_d": _bf16(np.ones(T, np.float32)),
                "wconst": _bf16(
                    np.concatenate([np.zeros(AR, np.float32), np.ones(1, np.float32)])
                ),
            }
        )
    return in_maps


def kernel(x, W_base, b_base, lora_A, lora_B, router_W, router_b):
    x = np.asarray(x, dtype=np.float32)
    W_base = np.asarray(W_base, dtype=np.float32)
    b_base = np.asarray(b_base, dtype=np.float32)
    lora_A = np.asarray(lora_A, dtype=np.float32)
    lora_B = np.asarray(lora_B, dtype=np.float32)
    router_W = np.asarray(router_W, dtype=np.float32)
    router_b = np.asarray(router_b, dtype=np.float32)

    B, S, D_ = x.shape
    O = W_base.shape[0]
    in_maps = _prep_in_maps(x, W_base, b_base, lora_A, lora_B, router_W, router_b)

    global _last_in_maps
    _last_in_maps = in_maps
    nc = _get_nc()
    res = run_bass_kernel_spmd(nc, in_maps, core_ids=list(range(8)))
    out = np.empty((B, S, O), dtype=np.float32)
    for c in range(8):
        b, half = c // 2, c % 2
        out[b, :, half * O_SH : (half + 1) * O_SH] = res.results[c]["out"].T
    return out
